# revision 60
# baseline (speedup 1.0000x reference)
"""Multi-head attention (RoPE, causal) Bass kernel for 8 TRN2 NeuronCores.

Sharding: 2-way batch x 4-way heads (4 heads per core); per-core partial
out[2048, 1024] summed on host (replaces the W_o-row-parallel AllReduce).

Fused schedule (per core): one software-pipelined stream per i-slice
"window".  Window `it` runs the attention of slice `it` — scores ->
exp (Act) -> causal mask (DVE) -> ctx matmul deferred one chunk-pair — and
pulls interleaved PE "bites" from a feed generator carrying the previous
slice's out-projection and the NEXT slice's QKV projections + RoPE, so the
PE never idles while Act chews the exps.  QT/KT stacked [128, S] bf16:
partitions 0-63 head even, 64-127 head odd.  Softmax denominator rides as
a 65th ones-column on V (PSUM row 64); 1/l via custom-DVE fast reciprocal
off a partition-0 staging row.  Everything bf16 except PSUM, RoPE tables
and the l/norm path (fp32).

PSUM (8 banks): tag "b" 2x[128,2,512] quads (4), tag "cx" 2x ctx
accumulators (2), tag "op" 2x shared proj/perm/vps/out-proj (2).
"""
import numpy as np
import ml_dtypes
from contextlib import ExitStack
from itertools import chain as ichain

import concourse.bass as bass
import concourse.tile as tile
from concourse import bacc, mybir
from concourse.bass_utils import run_bass_kernel_spmd

D_IN = 1024
D_OUT = 1024
HD = 64                   # head dim
S = 2048                  # sequence length
B = 2
THETA = 10000.0
NCORES = 8
IS = 512                  # i-slice width
NIS = S // IS             # 4 i-slices
NJC = S // 128            # 16 j-chunks

F32 = mybir.dt.float32
BF16 = mybir.dt.bfloat16
BF = ml_dtypes.bfloat16


def build_kernel():
    nc = bacc.Bacc("TRN2", target_bir_lowering=False, debug=False)

    # host pre-shuffled so every DMA is contiguous per partition:
    # xT[p, it, c, i] = x[b][512it+i, 128c+p]; w*[p, c, n] = W[128c+p, n]
    xT = nc.dram_tensor("xT", [128, NIS, 8, IS], BF16, kind="ExternalInput").ap()
    wq = nc.dram_tensor("wq", [128, 8, 256], BF16, kind="ExternalInput").ap()
    wk = nc.dram_tensor("wk", [128, 8, 256], BF16, kind="ExternalInput").ap()
    wv = nc.dram_tensor("wv", [128, 8, 256], BF16, kind="ExternalInput").ap()
    wo = nc.dram_tensor("wo", [128, 2, 1024], BF16, kind="ExternalInput").ap()
    cdup = nc.dram_tensor("cdup", [128, S], BF16, kind="ExternalInput").ap()
    sdup = nc.dram_tensor("sdup", [128, S], BF16, kind="ExternalInput").ap()
    p64 = nc.dram_tensor("p64", [128, 128], BF16, kind="ExternalInput").ap()
    sdup0 = nc.dram_tensor("sdup0", [128, IS], F32, kind="ExternalInput").ap()
    tri2 = nc.dram_tensor("tri2", [128, 2, 128], BF16, kind="ExternalInput").ap()
    onesc = nc.dram_tensor("onesc", [128, 65], BF16, kind="ExternalInput").ap()
    # out[p, it, ib, n] = row 512it+128ib+p of the [2048, 1024] partial
    out = nc.dram_tensor("out", [128, NIS, 4, 1024], BF16,
                         kind="ExternalOutput").ap()

    with tile.TileContext(nc) as tc, ExitStack() as ctx:
        singles = ctx.enter_context(tc.tile_pool(name="singles", bufs=1))
        xpool = ctx.enter_context(tc.tile_pool(name="xpool", bufs=2))
        rope_tmp = ctx.enter_context(tc.tile_pool(name="rope_tmp", bufs=4))
        expp = ctx.enter_context(tc.tile_pool(name="expp", bufs=5))
        bcp = ctx.enter_context(tc.tile_pool(name="bcp", bufs=2))
        ctxp = ctx.enter_context(tc.tile_pool(name="ctxp", bufs=3))
        outp = ctx.enter_context(tc.tile_pool(name="outp", bufs=2))
        psum = ctx.enter_context(tc.tile_pool(name="psum", bufs=2, space="PSUM"))

        # ---- DMAs, ordered by first PE use ----
        wq_t = singles.tile([128, 8, 256], BF16, tag="wq", name="wq")
        xt0_t = xpool.tile([128, 8, IS], BF16, tag="xt", name="xt0")
        for c in range(2):
            nc.sync.dma_start(out=wq_t[:, c, :], in_=wq[:, c, :])
            nc.sync.dma_start(out=xt0_t[:, c, :], in_=xT[:, 0, c, :])
        xts = {0: xt0_t}

        def xt_dma(it):
            t = xpool.tile([128, 8, IS], BF16, tag="xt", name=f"xt{it}")
            nc.sync.dma_start(out=t, in_=xT[:, it, :, :])
            xts[it] = t

        for c in range(2, 8):
            nc.sync.dma_start(out=wq_t[:, c, :], in_=wq[:, c, :])
            nc.sync.dma_start(out=xt0_t[:, c, :], in_=xT[:, 0, c, :])
        wk_t = singles.tile([128, 8, 256], BF16, tag="wk", name="wk")
        nc.sync.dma_start(out=wk_t[:, 0:4, :], in_=wk[:, 0:4, :])
        nc.sync.dma_start(out=wk_t[:, 4:8, :], in_=wk[:, 4:8, :])
        p64_sb = singles.tile([128, 128], BF16, tag="p64")
        nc.sync.dma_start(out=p64_sb, in_=p64)
        s0_sb = singles.tile([128, IS], F32, tag="sdup0")
        nc.sync.dma_start(out=s0_sb, in_=sdup0)
        c_sb = singles.tile([128, S], BF16, tag="cdup")
        nc.sync.dma_start(out=c_sb, in_=cdup)
        xt_dma(1)
        wv_t = singles.tile([128, 8, 256], BF16, tag="wv", name="wv")
        nc.sync.dma_start(out=wv_t, in_=wv)
        onesc_sb = singles.tile([128, 65], BF16, tag="ones")
        nc.sync.dma_start(out=onesc_sb, in_=onesc)
        tri2_sb = singles.tile([128, 2, 128], BF16, tag="tri2")
        nc.sync.dma_start(out=tri2_sb, in_=tri2)
        # sdup full table last: first used by window-0's slice-1 RoPE
        s_sb = singles.tile([128, S], BF16, tag="sdup")
        nc.sync.dma_start(out=s_sb, in_=sdup)
        # deferred: wo is not needed until the first out-projection
        wo_sb = singles.tile([128, 2, 1024], BF16, tag="wo")

        # persistent SBUF state
        qt = [singles.tile([128, S], BF16, tag=f"qt{p}", name=f"qt{p}")
              for p in range(2)]
        kt = [singles.tile([128, S], BF16, tag=f"kt{p}", name=f"kt{p}")
              for p in range(2)]
        v4 = singles.tile([128, NJC, 4, 65], BF16, tag="v4")
        v4_ones = bass.AP(tensor=v4.tensor, offset=64,
                          ap=[[NJC * 4 * 65, 128], [65, NJC * 4]])
        # 1/l rows, all on partition 0: [h, i] along the free dim.
        # lrow stages the PSUM row-64 l to partition 0 first: the custom-DVE
        # reciprocal mishandles nonzero input partition offsets.
        rl = singles.tile([128, 4, IS], F32, tag="rl")
        lrow = singles.tile([128, 4, IS], F32, tag="lrow")

        # ---------------- QKV projection + RoPE bites -----------------
        # rotate-half is a blockwise +-32 partition rotation: done with 4
        # SBUF->SBUF partition-range DMA copies instead of a PE perm matmul.
        def rope_stage1(proj, key, it):
            raw = rope_tmp.tile([128, IS], BF16, tag="raw", name=f"raw{key}")
            nc.scalar.copy(raw, proj)
            t1 = rope_tmp.tile([128, IS], BF16, tag="t1", name=f"t1{key}")
            nc.vector.tensor_mul(t1, raw, c_sb[:, it * IS:(it + 1) * IS])
            return raw, t1

        def rope_stage2(raw, t1, dest, it):
            if it == 0:
                # prologue: DMA queues are saturated with input loads and the
                # PE is DMA-starved, so rotate-half via a PE perm matmul
                perm = psum.tile([128, IS], F32, tag="op", name="perm")
                nc.tensor.matmul(perm, p64_sb, raw, start=True, stop=True)
                t2 = rope_tmp.tile([128, IS], BF16, tag="t2")
                nc.vector.tensor_mul(t2, perm, s0_sb)
                nc.vector.tensor_add(dest, t1, t2)
                return
            permr = rope_tmp.tile([128, IS], BF16, tag="pr", name="permr")
            for a, b in ((0, 32), (32, 0), (64, 96), (96, 64)):
                nc.sync.dma_start(out=permr[a:a + 32, :], in_=raw[b:b + 32, :])
            t2 = rope_tmp.tile([128, IS], BF16, tag="t2")
            nc.vector.tensor_mul(t2, permr, s_sb[:, it * IS:(it + 1) * IS])
            nc.vector.tensor_add(dest, t1, t2)

        def qkv_bites(it):
            """QKV projections + RoPE for slice `it`, yielded in ~2-matmul
            bites so they interleave into the attention pipeline."""
            if it >= NIS:
                return
            if 2 <= it + 1 < NIS:
                xt_dma(it + 1)
            xt_t = xts[it]
            pending_rope = None
            for tname, wt, dests in (("q", wq_t, qt), ("k", wk_t, kt)):
                for p in range(2):
                    proj = psum.tile([128, IS], F32, tag="op", name="proj")
                    for c in range(0, 8, 2):
                        nc.tensor.matmul(proj, wt[:, c, 128 * p:128 * (p + 1)],
                                         xt_t[:, c, :],
                                         start=(c == 0), stop=False)
                        nc.tensor.matmul(proj, wt[:, c + 1, 128 * p:128 * (p + 1)],
                                         xt_t[:, c + 1, :],
                                         start=False, stop=(c + 1 == 7))
                        yield
                    if pending_rope is not None:
                        rope_stage2(*pending_rope)
                    raw, t1 = rope_stage1(proj, f"{tname}{p}", it)
                    pending_rope = (raw, t1,
                                    dests[p][:, it * IS:(it + 1) * IS], it)
            for half in range(2):
                vps = psum.tile([128, 2, 256], F32, tag="op", name="vps")
                for js in range(2):
                    for c in range(0, 8, 2):
                        for cc in (c, c + 1):
                            nc.tensor.matmul(
                                vps[:, js, :],
                                xt_t[:, cc, 128 * (half * 2 + js):128 * (half * 2 + js + 1)],
                                wv_t[:, cc, :],
                                start=(cc == 0), stop=(cc == 7))
                        yield
                if half == 0 and pending_rope is not None:
                    rope_stage2(*pending_rope)
                    pending_rope = None
                for js in range(2):
                    jt = it * 4 + half * 2 + js
                    nc.vector.tensor_copy(v4[:, jt, :, 0:64],
                                          vps[:, js, :].rearrange("p (h d) -> p h d", h=4))

        def outproj_bites(ctxs_pair, it):
            """Out-projection of slice `it`: 8 bites of 2 matmuls each."""
            ot = outp.tile([128, 4, 1024], BF16, tag="o", name="ot")
            for ib in range(4):
                for nt in range(2):
                    ops = psum.tile([128, IS], F32, tag="op", name="ops")
                    for pair in range(2):
                        nc.tensor.matmul(
                            ops,
                            ctxs_pair[pair][:, 128 * ib:128 * (ib + 1)],
                            wo_sb[:, pair, nt * IS:(nt + 1) * IS],
                            start=(pair == 0), stop=(pair == 1))
                    # PSUM->SBUF copies on DVE: Act keeps only exp + raw
                    nc.vector.tensor_copy(ot[:, ib, nt * IS:(nt + 1) * IS], ops)
                    yield
                # per-ib DMA so the tail drains while later bites still run
                nc.sync.dma_start(out=out[:, it, ib, :], in_=ot[:, ib, :])

        # ---------------- prologue: slice-0 projections --------------
        for _ in qkv_bites(0):
            pass
        nc.sync.dma_start(out=wo_sb, in_=wo)
        # ones column for the softmax denominator
        nc.vector.tensor_copy(v4_ones, onesc_sb[:, 0:64])

        # ------------- fused attention windows ------------------------
        ctxs_pair = None
        feed = qkv_bites(1)

        # feed-bite budget per window, used to pace pulls evenly: window 3
        # has only the 8 out-proj bites for 32 pair-steps, so pull sparsely
        BITES = {0: 33, 1: 41, 2: 41, 3: 8}

        for it in range(NIS):
            njc = 4 * it + 4
            steps = 2 * (it + 1) * 4
            new_ctxs = [None, None]
            ctx_ps = {}
            pend = []
            step = 0

            def drain(it=it, njc=njc, new_ctxs=new_ctxs, ctx_ps=ctx_ps):
                h, q0, exps = pend.pop(0)
                for qi in range(2):
                    jc = q0 + qi
                    c0 = max(0, 128 * (jc - 4 * it))
                    nc.tensor.matmul(
                        ctx_ps[h][0:65, c0:IS],
                        v4[:, jc, h, :],
                        exps[:, qi, c0:IS],
                        start=(jc == 0), stop=(jc == njc - 1))
                if q0 + 2 == njc:
                    # head complete: 1/l, broadcast, normalize -> ctxs SBUF
                    pair, half = divmod(h, 2)
                    nc.vector.tensor_copy(lrow[0:1, h, :], ctx_ps[h][64:65, :])
                    nc.vector.reciprocal_approx_fast(rl[0:1, h, :],
                                                     lrow[0:1, h, :])
                    if half == 0:
                        new_ctxs[pair] = ctxp.tile([128, IS], BF16,
                                                   tag="c", name="ctxs")
                    bcs = bcp.tile([64, IS], F32, tag="bc", name="bcs")
                    nc.gpsimd.partition_broadcast(bcs, rl[0:1, h, :])
                    nc.vector.tensor_mul(
                        new_ctxs[pair][64 * half:64 * half + 64, :],
                        ctx_ps[h][0:64, :], bcs)

            for h in range(4):
                pair, half = divmod(h, 2)
                hb = 64 * half
                qs = qt[pair][hb:hb + 64, :]
                ks = kt[pair][hb:hb + 64, :]
                ctx_ps[h] = psum.tile([128, IS], F32, tag="cx", name=f"ctx{h}")
                for q0 in range(0, njc, 2):
                    quad = psum.tile([128, 2, IS], F32, tag="b", name="quad")
                    exps = expp.tile([128, 2, IS], BF16, tag="e", name="exps")
                    trim0 = max(0, 128 * (q0 - 4 * it))
                    for qi in range(2):
                        jc = q0 + qi
                        c0 = max(0, 128 * (jc - 4 * it))
                        nc.tensor.matmul(
                            quad[:, qi, c0:IS],
                            ks[:, 128 * jc:128 * (jc + 1)],
                            qs[:, it * IS + c0:(it + 1) * IS],
                            start=True, stop=True)
                    nc.scalar.activation(
                        exps[:, 0:2, trim0:IS], quad[:, 0:2, trim0:IS],
                        mybir.ActivationFunctionType.Exp, scale=0.125)
                    if q0 >= 4 * it:
                        # both chunks diagonal: one fused causal-mask multiply
                        m = bass.AP(tensor=exps.tensor,
                                    offset=exps.offset + trim0,
                                    ap=[exps.ap[0], [IS + 128, 2], [1, 128]])
                        nc.vector.tensor_mul(m, m, tri2_sb)
                    npull = (round((step + 1) * BITES[it] / steps)
                             - round(step * BITES[it] / steps))
                    for _ in range(npull):
                        next(feed, None)
                    step += 1
                    # window 3 is exp-co-bound with almost no feed: one
                    # extra pipeline stage absorbs the Act jitter there
                    if len(pend) >= (3 if it == 3 else 2):
                        drain()
                    pend.append((h, q0, exps))
            while pend:
                drain()
            for _ in feed:
                pass
            ctxs_pair = new_ctxs
            feed = ichain(outproj_bites(ctxs_pair, it), qkv_bites(it + 2))
        for _ in feed:
            pass

    nc.compile()
    return nc


def _host_tables():
    inv_freq = 1.0 / (THETA ** (np.arange(0, HD, 2, dtype=np.float64) / HD))
    pos = np.arange(S, dtype=np.float64)
    ang = pos[None, :] * inv_freq[:, None]          # [32, S]
    cos32 = np.cos(ang).astype(np.float32)
    sin32 = np.sin(ang).astype(np.float32)
    cdup = np.concatenate([cos32, cos32, cos32, cos32], axis=0)  # [128, S]
    s_signed = np.concatenate([-sin32, sin32, -sin32, sin32], axis=0)
    p64 = np.zeros((128, 128), dtype=np.float32)
    for m in range(128):
        blk = m - (m % 64)
        d = m % 64
        p64[blk + ((d + 32) % 64), m] = 1.0
    tri = (np.arange(128)[:, None] <= np.arange(128)[None, :]).astype(np.float32)
    tri2 = np.stack([tri, tri], axis=1)  # [128, 2, 128]
    return cdup, s_signed, p64, tri2


_NC_CACHE = {}


def make_in_maps(x, W_q, W_k, W_v, W_o):
    cdup, sdup, p64, tri2 = _host_tables()
    ones = np.ones((128, 65), dtype=BF)
    def wshuf(w):  # [1024, 256] -> [128, 8, 256]
        return np.ascontiguousarray(
            w.reshape(8, 128, 256).transpose(1, 0, 2)).astype(BF)

    in_maps = []
    for c in range(NCORES):
        b, g = divmod(c, 4)
        cols = slice(256 * g, 256 * (g + 1))
        # xT[p, it, c, i] = x[b][512it+i, 128c+p]
        xtr = np.ascontiguousarray(
            x[b].reshape(NIS, IS, 8, 128).transpose(3, 0, 2, 1)).astype(BF)
        in_maps.append({
            "xT": xtr,
            "wq": wshuf(W_q[:, cols]),
            "wk": wshuf(W_k[:, cols]),
            "wv": wshuf(W_v[:, cols]),
            "wo": np.ascontiguousarray(
                W_o[cols, :].reshape(2, 128, 1024).transpose(1, 0, 2)).astype(BF),
            "cdup": cdup.astype(BF), "sdup": sdup.astype(BF),
            "p64": p64.astype(BF),
            "sdup0": sdup[:, 0:IS].copy(),
            "tri2": tri2.astype(BF),
            "onesc": ones,
        })
    return in_maps


def kernel(x, W_q, W_k, W_v, W_o):
    x = np.ascontiguousarray(x, dtype=np.float32)
    W_q = np.ascontiguousarray(W_q, dtype=np.float32)
    W_k = np.ascontiguousarray(W_k, dtype=np.float32)
    W_v = np.ascontiguousarray(W_v, dtype=np.float32)
    W_o = np.ascontiguousarray(W_o, dtype=np.float32)

    if "nc" not in _NC_CACHE:
        _NC_CACHE["nc"] = build_kernel()
    nc = _NC_CACHE["nc"]

    in_maps = make_in_maps(x, W_q, W_k, W_v, W_o)
    res = run_bass_kernel_spmd(nc, in_maps, list(range(NCORES)))
    full = np.zeros((B, S, D_OUT), dtype=np.float32)
    for b in range(B):
        for g in range(4):
            arr = np.asarray(res.results[4 * b + g]["out"], dtype=np.float32)
            # arr[p, it, ib, n] -> row 512it+128ib+p
            full[b] += arr.transpose(1, 2, 0, 3).reshape(S, D_OUT)
    return full


# revision 62
# speedup vs baseline: 1.1556x; 1.1556x over previous
"""Multi-head attention (RoPE, causal) Bass kernel for 8 TRN2 NeuronCores.

Sharding: 2-way batch x 4-way heads (4 heads per core); per-core partial
out[2048, 1024] summed on host (replaces the W_o-row-parallel AllReduce).

Fused schedule (per core): one software-pipelined stream per i-slice
"window".  Window `it` runs the attention of slice `it` — scores ->
exp (Act) -> causal mask (DVE) -> ctx matmul deferred one chunk-pair — and
pulls interleaved PE "bites" from a feed generator carrying the previous
slice's out-projection and the NEXT slice's QKV projections + RoPE, so the
PE never idles while Act chews the exps.  QT/KT stacked [128, S] bf16:
partitions 0-63 head even, 64-127 head odd.  Softmax denominator rides as
a 65th ones-column on V (PSUM row 64); 1/l via custom-DVE fast reciprocal
off a partition-0 staging row.  Everything bf16 except PSUM, RoPE tables
and the l/norm path (fp32).

PSUM (8 banks): tag "b" 2x[128,2,512] quads (4), tag "cx" 2x ctx
accumulators (2), tag "op" 2x shared proj/perm/vps/out-proj (2).
"""
import numpy as np
import ml_dtypes
from contextlib import ExitStack
from itertools import chain as ichain

import concourse.bass as bass
import concourse.tile as tile
from concourse import bacc, mybir
from concourse.bass_utils import run_bass_kernel_spmd

D_IN = 1024
D_OUT = 1024
HD = 64                   # head dim
S = 2048                  # sequence length
B = 2
THETA = 10000.0
NCORES = 8
IS = 512                  # i-slice width
NIS = S // IS             # 4 i-slices
NJC = S // 128            # 16 j-chunks

F32 = mybir.dt.float32
BF16 = mybir.dt.bfloat16
BF = ml_dtypes.bfloat16


def build_kernel():
    nc = bacc.Bacc("TRN2", target_bir_lowering=False, debug=False)

    # host pre-shuffled so every DMA is contiguous per partition:
    # xT[p, it, c, i] = x[b][512it+i, 128c+p]; w*[p, c, n] = W[128c+p, n]
    xT = nc.dram_tensor("xT", [128, NIS, 8, IS], BF16, kind="ExternalInput").ap()
    wq = nc.dram_tensor("wq", [128, 8, 256], BF16, kind="ExternalInput").ap()
    wk = nc.dram_tensor("wk", [128, 8, 256], BF16, kind="ExternalInput").ap()
    wv = nc.dram_tensor("wv", [128, 8, 256], BF16, kind="ExternalInput").ap()
    wo = nc.dram_tensor("wo", [128, 2, 1024], BF16, kind="ExternalInput").ap()
    cdup = nc.dram_tensor("cdup", [128, S], BF16, kind="ExternalInput").ap()
    sdup = nc.dram_tensor("sdup", [128, S], BF16, kind="ExternalInput").ap()
    p64 = nc.dram_tensor("p64", [128, 128], BF16, kind="ExternalInput").ap()
    sdup0 = nc.dram_tensor("sdup0", [128, IS], F32, kind="ExternalInput").ap()
    tri2 = nc.dram_tensor("tri2", [128, 2, 128], BF16, kind="ExternalInput").ap()
    onesc = nc.dram_tensor("onesc", [128, 65], BF16, kind="ExternalInput").ap()
    # out[p, it, ib, n] = row 512it+128ib+p of the [2048, 1024] partial
    out = nc.dram_tensor("out", [128, NIS, 4, 1024], BF16,
                         kind="ExternalOutput").ap()

    with tile.TileContext(nc) as tc, ExitStack() as ctx:
        singles = ctx.enter_context(tc.tile_pool(name="singles", bufs=1))
        xpool = ctx.enter_context(tc.tile_pool(name="xpool", bufs=2))
        rope_tmp = ctx.enter_context(tc.tile_pool(name="rope_tmp", bufs=4))
        expp = ctx.enter_context(tc.tile_pool(name="expp", bufs=5))
        bcp = ctx.enter_context(tc.tile_pool(name="bcp", bufs=2))
        ctxp = ctx.enter_context(tc.tile_pool(name="ctxp", bufs=3))
        outp = ctx.enter_context(tc.tile_pool(name="outp", bufs=2))
        psum = ctx.enter_context(tc.tile_pool(name="psum", bufs=2, space="PSUM"))

        # ---- DMAs, ordered by first PE use ----
        wq_t = singles.tile([128, 8, 256], BF16, tag="wq", name="wq")
        xt0_t = xpool.tile([128, 8, IS], BF16, tag="xt", name="xt0")
        for c in range(2):
            nc.sync.dma_start(out=wq_t[:, c, :], in_=wq[:, c, :])
            nc.sync.dma_start(out=xt0_t[:, c, :], in_=xT[:, 0, c, :])
        xts = {0: xt0_t}

        def xt_dma(it):
            t = xpool.tile([128, 8, IS], BF16, tag="xt", name=f"xt{it}")
            nc.sync.dma_start(out=t, in_=xT[:, it, :, :])
            xts[it] = t

        for c in range(2, 8):
            nc.sync.dma_start(out=wq_t[:, c, :], in_=wq[:, c, :])
            nc.sync.dma_start(out=xt0_t[:, c, :], in_=xT[:, 0, c, :])
        wk_t = singles.tile([128, 8, 256], BF16, tag="wk", name="wk")
        nc.sync.dma_start(out=wk_t[:, 0:4, :], in_=wk[:, 0:4, :])
        nc.sync.dma_start(out=wk_t[:, 4:8, :], in_=wk[:, 4:8, :])
        p64_sb = singles.tile([128, 128], BF16, tag="p64")
        nc.sync.dma_start(out=p64_sb, in_=p64)
        s0_sb = singles.tile([128, IS], F32, tag="sdup0")
        nc.sync.dma_start(out=s0_sb, in_=sdup0)
        c_sb = singles.tile([128, S], BF16, tag="cdup")
        nc.sync.dma_start(out=c_sb, in_=cdup)
        xt_dma(1)
        wv_t = singles.tile([128, 8, 256], BF16, tag="wv", name="wv")
        nc.sync.dma_start(out=wv_t, in_=wv)
        onesc_sb = singles.tile([128, 65], BF16, tag="ones")
        nc.sync.dma_start(out=onesc_sb, in_=onesc)
        tri2_sb = singles.tile([128, 2, 128], BF16, tag="tri2")
        nc.sync.dma_start(out=tri2_sb, in_=tri2)
        # sdup full table last: first used by window-0's slice-1 RoPE
        s_sb = singles.tile([128, S], BF16, tag="sdup")
        nc.sync.dma_start(out=s_sb, in_=sdup)
        # deferred: wo is not needed until the first out-projection
        wo_sb = singles.tile([128, 2, 1024], BF16, tag="wo")

        # persistent SBUF state
        qt = [singles.tile([128, S], BF16, tag=f"qt{p}", name=f"qt{p}")
              for p in range(2)]
        kt = [singles.tile([128, S], BF16, tag=f"kt{p}", name=f"kt{p}")
              for p in range(2)]
        v4 = singles.tile([128, NJC, 4, 65], BF16, tag="v4")
        v4_ones = bass.AP(tensor=v4.tensor, offset=64,
                          ap=[[NJC * 4 * 65, 128], [65, NJC * 4]])
        # 1/l rows, all on partition 0: [h, i] along the free dim.
        # lrow stages the PSUM row-64 l to partition 0 first: the custom-DVE
        # reciprocal mishandles nonzero input partition offsets.
        rl = singles.tile([128, 4, IS], F32, tag="rl")
        lrow = singles.tile([128, 4, IS], F32, tag="lrow")

        # ---------------- QKV projection + RoPE bites -----------------
        # rotate-half is a blockwise +-32 partition rotation: done with 4
        # SBUF->SBUF partition-range DMA copies instead of a PE perm matmul.
        def rope_stage1(proj, key, it):
            raw = rope_tmp.tile([128, IS], BF16, tag="raw", name=f"raw{key}")
            nc.scalar.copy(raw, proj)
            t1 = rope_tmp.tile([128, IS], BF16, tag="t1", name=f"t1{key}")
            nc.vector.tensor_mul(t1, raw, c_sb[:, it * IS:(it + 1) * IS])
            return raw, t1

        def rope_stage2(raw, t1, dest, it):
            if it == 0:
                # prologue: DMA queues are saturated with input loads and the
                # PE is DMA-starved, so rotate-half via a PE perm matmul
                perm = psum.tile([128, IS], F32, tag="op", name="perm")
                nc.tensor.matmul(perm, p64_sb, raw, start=True, stop=True)
                t2 = rope_tmp.tile([128, IS], BF16, tag="t2")
                nc.vector.tensor_mul(t2, perm, s0_sb)
                nc.vector.tensor_add(dest, t1, t2)
                return
            permr = rope_tmp.tile([128, IS], BF16, tag="pr", name="permr")
            for a, b in ((0, 32), (32, 0), (64, 96), (96, 64)):
                nc.sync.dma_start(out=permr[a:a + 32, :], in_=raw[b:b + 32, :])
            t2 = rope_tmp.tile([128, IS], BF16, tag="t2")
            nc.vector.tensor_mul(t2, permr, s_sb[:, it * IS:(it + 1) * IS])
            nc.vector.tensor_add(dest, t1, t2)

        def qkv_bites(it):
            """QKV projections + RoPE for slice `it`, yielded in ~2-matmul
            bites so they interleave into the attention pipeline."""
            if it >= NIS:
                return
            if 2 <= it + 1 < NIS:
                xt_dma(it + 1)
            xt_t = xts[it]
            pending_rope = None
            for tname, wt, dests in (("q", wq_t, qt), ("k", wk_t, kt)):
                for p in range(2):
                    proj = psum.tile([128, IS], F32, tag="op", name="proj")
                    for c in range(0, 8, 2):
                        nc.tensor.matmul(proj, wt[:, c, 128 * p:128 * (p + 1)],
                                         xt_t[:, c, :],
                                         start=(c == 0), stop=False)
                        nc.tensor.matmul(proj, wt[:, c + 1, 128 * p:128 * (p + 1)],
                                         xt_t[:, c + 1, :],
                                         start=False, stop=(c + 1 == 7))
                        yield
                    if pending_rope is not None:
                        rope_stage2(*pending_rope)
                    raw, t1 = rope_stage1(proj, f"{tname}{p}", it)
                    pending_rope = (raw, t1,
                                    dests[p][:, it * IS:(it + 1) * IS], it)
            for half in range(2):
                vps = psum.tile([128, 2, 256], F32, tag="op", name="vps")
                for js in range(2):
                    for c in range(0, 8, 2):
                        for cc in (c, c + 1):
                            nc.tensor.matmul(
                                vps[:, js, :],
                                xt_t[:, cc, 128 * (half * 2 + js):128 * (half * 2 + js + 1)],
                                wv_t[:, cc, :],
                                start=(cc == 0), stop=(cc == 7))
                        yield
                if half == 0 and pending_rope is not None:
                    rope_stage2(*pending_rope)
                    pending_rope = None
                for js in range(2):
                    jt = it * 4 + half * 2 + js
                    nc.vector.tensor_copy(v4[:, jt, :, 0:64],
                                          vps[:, js, :].rearrange("p (h d) -> p h d", h=4))

        def outproj_bites(ctxs_pair, it):
            """Out-projection of slice `it`: 8 bites of 2 matmuls each."""
            ot = outp.tile([128, 4, 1024], BF16, tag="o", name="ot")
            for ib in range(4):
                for nt in range(2):
                    ops = psum.tile([128, IS], F32, tag="op", name="ops")
                    for pair in range(2):
                        nc.tensor.matmul(
                            ops,
                            ctxs_pair[pair][:, 128 * ib:128 * (ib + 1)],
                            wo_sb[:, pair, nt * IS:(nt + 1) * IS],
                            start=(pair == 0), stop=(pair == 1))
                    # PSUM->SBUF copies on DVE: Act keeps only exp + raw
                    nc.vector.tensor_copy(ot[:, ib, nt * IS:(nt + 1) * IS], ops)
                    yield
                # per-ib DMA so the tail drains while later bites still run
                nc.sync.dma_start(out=out[:, it, ib, :], in_=ot[:, ib, :])

        # ---------------- prologue: slice-0 projections --------------
        for _ in qkv_bites(0):
            pass
        nc.sync.dma_start(out=wo_sb, in_=wo)
        # ones column for the softmax denominator
        nc.vector.tensor_copy(v4_ones, onesc_sb[:, 0:64])

        # ------------- fused attention windows ------------------------
        ctxs_pair = None
        feed = qkv_bites(1)

        # feed-bite budget per window, used to pace pulls evenly: window 3
        # has only the 8 out-proj bites for 32 pair-steps, so pull sparsely
        BITES = {0: 33, 1: 41, 2: 41, 3: 8}

        for it in range(NIS):
            njc = 4 * it + 4
            steps = 2 * (it + 1) * 4
            new_ctxs = [None, None]
            ctx_ps = {}
            pend = []
            step = 0

            def drain(it=it, njc=njc, new_ctxs=new_ctxs, ctx_ps=ctx_ps):
                h, q0, exps = pend.pop(0)
                for qi in range(2):
                    jc = q0 + qi
                    c0 = max(0, 128 * (jc - 4 * it))
                    nc.tensor.matmul(
                        ctx_ps[h][0:65, c0:IS],
                        v4[:, jc, h, :],
                        exps[:, qi, c0:IS],
                        start=(jc == 0), stop=(jc == njc - 1))
                if q0 + 2 == njc:
                    # head complete: 1/l, broadcast, normalize -> ctxs SBUF
                    pair, half = divmod(h, 2)
                    nc.vector.tensor_copy(lrow[0:1, h, :], ctx_ps[h][64:65, :])
                    nc.vector.reciprocal_approx_fast(rl[0:1, h, :],
                                                     lrow[0:1, h, :])
                    if half == 0:
                        new_ctxs[pair] = ctxp.tile([128, IS], BF16,
                                                   tag="c", name="ctxs")
                    bcs = bcp.tile([64, IS], F32, tag="bc", name="bcs")
                    nc.gpsimd.partition_broadcast(bcs, rl[0:1, h, :])
                    nc.vector.tensor_mul(
                        new_ctxs[pair][64 * half:64 * half + 64, :],
                        ctx_ps[h][0:64, :], bcs)

            for h in range(4):
                pair, half = divmod(h, 2)
                hb = 64 * half
                qs = qt[pair][hb:hb + 64, :]
                ks = kt[pair][hb:hb + 64, :]
                ctx_ps[h] = psum.tile([128, IS], F32, tag="cx", name=f"ctx{h}")
                for q0 in range(0, njc, 2):
                    quad = psum.tile([128, 2, IS], F32, tag="b", name="quad")
                    exps = expp.tile([128, 2, IS], BF16, tag="e", name="exps")
                    trim0 = max(0, 128 * (q0 - 4 * it))
                    for qi in range(2):
                        jc = q0 + qi
                        c0 = max(0, 128 * (jc - 4 * it))
                        nc.tensor.matmul(
                            quad[:, qi, c0:IS],
                            ks[:, 128 * jc:128 * (jc + 1)],
                            qs[:, it * IS + c0:(it + 1) * IS],
                            start=True, stop=True)
                    nc.scalar.activation(
                        exps[:, 0:2, trim0:IS], quad[:, 0:2, trim0:IS],
                        mybir.ActivationFunctionType.Exp, scale=0.125)
                    if q0 >= 4 * it:
                        # both chunks diagonal: one fused causal-mask multiply
                        m = bass.AP(tensor=exps.tensor,
                                    offset=exps.offset + trim0,
                                    ap=[exps.ap[0], [IS + 128, 2], [1, 128]])
                        nc.vector.tensor_mul(m, m, tri2_sb)
                    npull = (round((step + 1) * BITES[it] / steps)
                             - round(step * BITES[it] / steps))
                    for _ in range(npull):
                        next(feed, None)
                    step += 1
                    if len(pend) >= 2:
                        drain()
                    pend.append((h, q0, exps))
            # leftover feed first: it carries no exp dependencies, so the PE
            # chews it while Act finishes the window's last exps, and the
            # final drains then issue without stalling the queue head
            for _ in feed:
                pass
            while pend:
                drain()
            ctxs_pair = new_ctxs
            feed = ichain(outproj_bites(ctxs_pair, it), qkv_bites(it + 2))
        for _ in feed:
            pass

    nc.compile()
    return nc


def _host_tables():
    inv_freq = 1.0 / (THETA ** (np.arange(0, HD, 2, dtype=np.float64) / HD))
    pos = np.arange(S, dtype=np.float64)
    ang = pos[None, :] * inv_freq[:, None]          # [32, S]
    cos32 = np.cos(ang).astype(np.float32)
    sin32 = np.sin(ang).astype(np.float32)
    cdup = np.concatenate([cos32, cos32, cos32, cos32], axis=0)  # [128, S]
    s_signed = np.concatenate([-sin32, sin32, -sin32, sin32], axis=0)
    p64 = np.zeros((128, 128), dtype=np.float32)
    for m in range(128):
        blk = m - (m % 64)
        d = m % 64
        p64[blk + ((d + 32) % 64), m] = 1.0
    tri = (np.arange(128)[:, None] <= np.arange(128)[None, :]).astype(np.float32)
    tri2 = np.stack([tri, tri], axis=1)  # [128, 2, 128]
    return cdup, s_signed, p64, tri2


_NC_CACHE = {}


def make_in_maps(x, W_q, W_k, W_v, W_o):
    cdup, sdup, p64, tri2 = _host_tables()
    ones = np.ones((128, 65), dtype=BF)
    def wshuf(w):  # [1024, 256] -> [128, 8, 256]
        return np.ascontiguousarray(
            w.reshape(8, 128, 256).transpose(1, 0, 2)).astype(BF)

    in_maps = []
    for c in range(NCORES):
        b, g = divmod(c, 4)
        cols = slice(256 * g, 256 * (g + 1))
        # xT[p, it, c, i] = x[b][512it+i, 128c+p]
        xtr = np.ascontiguousarray(
            x[b].reshape(NIS, IS, 8, 128).transpose(3, 0, 2, 1)).astype(BF)
        in_maps.append({
            "xT": xtr,
            "wq": wshuf(W_q[:, cols]),
            "wk": wshuf(W_k[:, cols]),
            "wv": wshuf(W_v[:, cols]),
            "wo": np.ascontiguousarray(
                W_o[cols, :].reshape(2, 128, 1024).transpose(1, 0, 2)).astype(BF),
            "cdup": cdup.astype(BF), "sdup": sdup.astype(BF),
            "p64": p64.astype(BF),
            "sdup0": sdup[:, 0:IS].copy(),
            "tri2": tri2.astype(BF),
            "onesc": ones,
        })
    return in_maps


def kernel(x, W_q, W_k, W_v, W_o):
    x = np.ascontiguousarray(x, dtype=np.float32)
    W_q = np.ascontiguousarray(W_q, dtype=np.float32)
    W_k = np.ascontiguousarray(W_k, dtype=np.float32)
    W_v = np.ascontiguousarray(W_v, dtype=np.float32)
    W_o = np.ascontiguousarray(W_o, dtype=np.float32)

    if "nc" not in _NC_CACHE:
        _NC_CACHE["nc"] = build_kernel()
    nc = _NC_CACHE["nc"]

    in_maps = make_in_maps(x, W_q, W_k, W_v, W_o)
    res = run_bass_kernel_spmd(nc, in_maps, list(range(NCORES)))
    full = np.zeros((B, S, D_OUT), dtype=np.float32)
    for b in range(B):
        for g in range(4):
            arr = np.asarray(res.results[4 * b + g]["out"], dtype=np.float32)
            # arr[p, it, ib, n] -> row 512it+128ib+p
            full[b] += arr.transpose(1, 2, 0, 3).reshape(S, D_OUT)
    return full
